# revision 15
# baseline (speedup 1.0000x reference)
import numpy as np
import jax
import jax.numpy as jnp
from jax.sharding import Mesh, PartitionSpec, NamedSharding
from jax.experimental.shard_map import shard_map

import concourse.tile as tile
from concourse import bacc, mybir
from concourse import bass2jax
from concourse.bass2jax import _bass_exec_p, install_neuronx_cc_hook

E, H, D = 128, 8, 16
QDIM, DYN, HID = 16, 3, 64
CLIP = 10.0
NCORES = 8
B, P, N = 16, 100, 1000
NB = 1                     # batches per core per launch
CH = B // (NCORES * NB)    # launches (chunks) per call
NT = (N + 127) // 128      # n tiles of 128 (last = 104)
LASTN = N - 128 * (NT - 1)

f32 = mybir.dt.float32
f16 = mybir.dt.float16
bf16 = mybir.dt.bfloat16
AF = mybir.ActivationFunctionType
ALU = mybir.AluOpType

# f16-sourced weights upconverted to f32 on device
W16 = ("wq2", "fw1a", "fw2a", "wk", "wv", "wcomb")


def _build_nc():
    nc = bacc.Bacc("TRN2", target_bir_lowering=False, debug=False,
                   num_devices=NCORES)

    def din(name, shape, dt=f32):
        return nc.dram_tensor(name, shape, dt, kind="ExternalInput").ap()

    xt = din("xt", [NB, 25, 24, 500], f16)
    nodes = din("nodes", [NB, N, E], f16)
    last = din("last", [NB, P, E], f16)
    loadv = din("loadv", [NB, 1, P])
    ninf = din("ninf", [NB, P, N], f16)
    wl1t = din("wl1t", [4, 24, E], f16)   # L1 templates (dyn rows only)
    w2d = din("w2d", [E, E], f16)
    w3d = din("w3d", [E, 32], f16)
    wq2 = din("wq2", [E, E], f16)
    wq2b = din("wq2b", [1, E])
    fw1a = din("fw1a", [E, E], f16)
    fw1b = din("fw1b", [1, E])
    fw1c = din("fw1c", [E, 1])
    fw1d = din("fw1d", [1, 1])
    fw2a = din("fw2a", [E, E], f16)
    fw2b = din("fw2b", [1, E])
    fb1 = din("fb1", [E, 1])
    fb1e = din("fb1e", [1, 1])
    fb2 = din("fb2", [E, 1])
    lqw = din("lqw", [E, QDIM])
    lqwb = din("lqwb", [1, QDIM])
    lqb = din("lqb", [QDIM, 1])
    lmw1q = din("lmw1q", [QDIM, HID])
    lmb1 = din("lmb1", [HID, 1])
    b2b2 = din("b2b2", [E, 1])
    nb3 = din("nb3", [E, 1])
    wk = din("wk", [E, E], f16)
    wv = din("wv", [E, E], f16)
    wcomb = din("wcomb", [E, E], f16)
    bcomb = din("bcomb", [E, 1])
    identm = din("identm", [E, E], f16)
    probs = nc.dram_tensor("probs", [NB, P, N], bf16, kind="ExternalOutput").ap()

    with tile.TileContext(nc) as tc:
        with (
            tc.tile_pool(name="wpool", bufs=1) as wp,
            tc.tile_pool(name="perb", bufs=1) as pb,
            tc.tile_pool(name="sball", bufs=3) as sb,
            tc.tile_pool(name="ps_ptr", bufs=2, space="PSUM") as pptr,
        ):
            def wtile(shape, dt, src, tag):
                t = wp.tile(shape, dt, tag=tag, name=tag)
                nc.sync.dma_start(t[:], src)
                return t

            identh = wtile([E, E], f16, identm, "identh")
            identd = wp.tile([E, E], f32, tag="identd", name="identd")
            nc.scalar.copy(identd[:], identh[:])
            w2d_sb = wtile([E, E], f16, w2d, "w2d")
            w3d_sb = wtile([E, 32], f16, w3d, "w3d")
            w1l = [wtile([24, E], f16, wl1t[s], f"w1l{s}") for s in range(4)]
            # big f32 weights arrive as f16, upconvert once on device
            w16sb = {}
            for nm, src in (("wq2", wq2), ("fw1a", fw1a), ("fw2a", fw2a),
                            ("wk", wk), ("wv", wv), ("wcomb", wcomb)):
                t16 = wtile([E, E], f16, src, nm + "_16")
                t32 = wp.tile([E, E], f32, tag=nm, name=nm)
                nc.scalar.copy(t32[:], t16[:])
                w16sb[nm] = t32
            wq2_sb = w16sb["wq2"]
            fw1a_sb = w16sb["fw1a"]
            fw2a_sb = w16sb["fw2a"]
            wk_sb = w16sb["wk"]
            wv_sb = w16sb["wv"]
            wcomb_sb = w16sb["wcomb"]
            wq2b_sb = wtile([1, E], f32, wq2b, "wq2b")
            fw1b_sb = wtile([1, E], f32, fw1b, "fw1b")
            fw1c_sb = wtile([E, 1], f32, fw1c, "fw1c")
            fw1d_sb = wtile([1, 1], f32, fw1d, "fw1d")
            fw2b_sb = wtile([1, E], f32, fw2b, "fw2b")
            fb1_sb = wtile([E, 1], f32, fb1, "fb1")
            fb1e_sb = wtile([1, 1], f32, fb1e, "fb1e")
            fb2_sb = wtile([E, 1], f32, fb2, "fb2")
            lqw_sb = wtile([E, QDIM], f32, lqw, "lqw")
            lqwb_sb = wtile([1, QDIM], f32, lqwb, "lqwb")
            lqb_sb = wtile([QDIM, 1], f32, lqb, "lqb")
            lmw1q_sb = wtile([QDIM, HID], f32, lmw1q, "lmw1q")
            lmb1_sb = wtile([HID, 1], f32, lmb1, "lmb1")
            b2b2_sb = wtile([E, 1], f32, b2b2, "b2b2")
            nb3_sb = wtile([E, 1], f32, nb3, "nb3")
            bcomb_sb = wtile([E, 1], f32, bcomb, "bcomb")
            ones_sb = wp.tile([E, 1], f32, tag="ones", name="ones")
            nc.vector.memset(ones_sb[:], 1.0)

            ndT = [pb.tile([E, 1024], f32, tag=f"ndT{b}", name=f"ndT{b}") for b in range(NB)]
            kTt = [pb.tile([E, 1024], f32, tag=f"kT{b}", name=f"kT{b}") for b in range(NB)]
            vsb = [pb.tile([E, NT * E], f32, tag=f"v{b}", name=f"v{b}") for b in range(NB)]
            qTt = [pb.tile([E, P], f32, tag=f"qT{b}", name=f"qT{b}") for b in range(NB)]
            qho = [[pb.tile([D, P], f32, tag=f"qh{b}{h}", name=f"qh{b}{h}")
                    for h in range(1, H, 2)] for b in range(NB)]
            kho = [[pb.tile([D, 1024], f32, tag=f"kh{b}{h}", name=f"kh{b}{h}")
                    for h in range(1, H, 2)] for b in range(NB)]
            Gt = [pb.tile([P, N], f32, tag=f"G{b}", name=f"G{b}") for b in range(NB)]
            GTt = [pb.tile([E, NT * P], f32, tag=f"GT{b}", name=f"GT{b}") for b in range(NB)]
            mhT = [pb.tile([E, P], f32, tag=f"mh{b}", name=f"mh{b}") for b in range(NB)]
            enT = [pb.tile([P, N], f32, tag=f"en{b}", name=f"en{b}") for b in range(NB)]
            c1b = [pb.tile([E, P], f32, tag=f"c1b{b}", name=f"c1b{b}") for b in range(NB)]

            # ============ PHASE 1: prep + lazy-mask MLP ============
            with (
                tc.tile_pool(name="ps_mlp", bufs=3, space="PSUM") as pmlp,
                tc.tile_pool(name="ps_s3", bufs=2, space="PSUM") as ps3,
                tc.tile_pool(name="xtp", bufs=3) as xtp,
                tc.tile_pool(name="mlpact", bufs=3) as ma,
            ):
                for b in range(NB):
                    nc.vector.memset(ndT[b][:, N:1024], 0.0)
                    nc.vector.memset(kTt[b][:, N:1024], 0.0)
                    # nodes^T
                    for half in range(2):
                        ps = pptr.tile([E, 512], f16, tag="ptr", name="ps")
                        for c in range(4):
                            t = half * 4 + c
                            rows = LASTN if t == NT - 1 else 128
                            nd_in = ma.tile([128, E], f16, tag="ndin", name="nd_in")
                            nc.sync.dma_start(nd_in[:rows, :],
                                              nodes[b, t * 128:t * 128 + rows, :])
                            nc.tensor.transpose(ps[:, c * 128:c * 128 + rows],
                                                nd_in[:rows, :], identh[:rows, :rows])
                        w = 512 if half == 0 else N - 512
                        nc.scalar.copy(ndT[b][:, half * 512:half * 512 + w], ps[:, :w])
                    # kT
                    for half in range(2):
                        ps = pptr.tile([E, 512], f32, tag="ptr", name="ps")
                        nc.tensor.matmul(ps[:, :500], wk_sb[:],
                                         ndT[b][:, half * 500:half * 500 + 500],
                                         start=True, stop=True)
                        nc.scalar.copy(kTt[b][:, half * 500:half * 500 + 500], ps[:, :500])
                    for i, h in enumerate(range(1, H, 2)):
                        nc.sync.dma_start(kho[b][i][:], kTt[b][h * D:(h + 1) * D, :])
                    # v natural
                    for half in range(2):
                        ps = pptr.tile([E, 512], f32, tag="ptr", name="ps")
                        for c in range(4):
                            t = half * 4 + c
                            nc.tensor.matmul(ps[:, c * 128:c * 128 + 128],
                                             ndT[b][:, t * 128:t * 128 + 128],
                                             wv_sb[:], start=True, stop=True)
                        nc.scalar.copy(vsb[b][:, half * 512:half * 512 + 512], ps[:])
                    # q path
                    la_in = ma.tile([P, E], f16, tag="lain", name="la_in")
                    nc.sync.dma_start(la_in[:], last[b])
                    lo_in = ma.tile([1, P], f32, tag="loin", name="lo_in")
                    nc.sync.dma_start(lo_in[:], loadv[b])
                    ps = pptr.tile([E, 512], f16, tag="ptr", name="ps")
                    nc.tensor.transpose(ps[:, :P], la_in[:], identh[:P, :P])
                    laT = ma.tile([E, P], f32, tag="laT", name="laT")
                    nc.scalar.copy(laT[:], ps[:, :P])

                    ps = pptr.tile([E, 512], f32, tag="ptr", name="ps")
                    nc.tensor.matmul(ps[:, :P], fw1a_sb[:], laT[:], start=True, stop=False)
                    nc.tensor.matmul(ps[:, :P], fw1b_sb[:], lo_in[:], start=False, stop=True)
                    r128 = ma.tile([E, P], f32, tag="r128", name="r128")
                    nc.scalar.activation(r128[:], ps[:, :P], AF.Relu, bias=fb1_sb[:, 0:1])
                    ps = pptr.tile([E, 512], f32, tag="ptr", name="ps")
                    nc.tensor.matmul(ps[:1, :P], fw1c_sb[:], laT[:], start=True, stop=False)
                    nc.tensor.matmul(ps[:1, :P], fw1d_sb[:], lo_in[:], start=False, stop=True)
                    rl = ma.tile([1, P], f32, tag="rl", name="rl")
                    nc.scalar.activation(rl[:], ps[:1, :P], AF.Relu, bias=fb1e_sb[:, 0:1])

                    ps = pptr.tile([E, 512], f32, tag="ptr", name="ps")
                    nc.tensor.matmul(ps[:, :P], fw2a_sb[:], r128[:], start=True, stop=False)
                    nc.tensor.matmul(ps[:, :P], fw2b_sb[:], rl[:], start=False, stop=True)
                    sig = ma.tile([E, P], f32, tag="sig", name="sig")
                    nc.scalar.activation(sig[:], ps[:, :P], AF.Sigmoid, bias=fb2_sb[:, 0:1])

                    ps = pptr.tile([E, 512], f32, tag="ptr", name="ps")
                    nc.tensor.matmul(ps[:, :P], wq2_sb[:], laT[:], start=True, stop=False)
                    nc.tensor.matmul(ps[:, :P], wq2b_sb[:], lo_in[:], start=False, stop=True)
                    qraw = ma.tile([E, P], f32, tag="qraw", name="qraw")
                    nc.scalar.copy(qraw[:], ps[:, :P])
                    nc.vector.tensor_mul(qTt[b][:], qraw[:], sig[:])
                    for i, h in enumerate(range(1, H, 2)):
                        nc.sync.dma_start(qho[b][i][:], qTt[b][h * D:(h + 1) * D, :])

                    ps = pptr.tile([E, 512], f32, tag="ptr", name="ps")
                    nc.tensor.matmul(ps[:QDIM, :P], lqw_sb[:], laT[:], start=True, stop=False)
                    nc.tensor.matmul(ps[:QDIM, :P], lqwb_sb[:], lo_in[:], start=False, stop=True)
                    qfT = ma.tile([QDIM, P], f32, tag="qfT", name="qfT")
                    nc.scalar.activation(qfT[:], ps[:QDIM, :P], AF.Identity, bias=lqb_sb[:, 0:1])
                    ps = pptr.tile([E, 512], f32, tag="ptr", name="ps")
                    nc.tensor.matmul(ps[:HID, :P], lmw1q_sb[:], qfT[:], start=True, stop=True)
                    c1s = ma.tile([HID, P], f32, tag="c1s", name="c1s")
                    nc.scalar.activation(c1s[:], ps[:HID, :P], AF.Identity, bias=lmb1_sb[:, 0:1])
                    nc.sync.dma_start(c1b[b][0:HID, :], c1s[:])
                    nc.sync.dma_start(c1b[b][HID:E, :], c1s[:])

                    # lazy MLP
                    for g in range(25):
                        xg = xtp.tile([24, 500], f16, tag="xg", name="xg")
                        nc.sync.dma_start(xg[:], xt[b, g])
                        s3ps = ps3.tile([E, 512], f32, tag="s3", name="s3ps")
                        for s in range(4):
                            p = 4 * g + s
                            h1ps = pmlp.tile([E, 500], f32, tag="mm", name="h1ps")
                            nc.tensor.matmul(h1ps[:], w1l[s][:], xg[:], start=True, stop=True)
                            h1sb = ma.tile([E, 500], f16, tag="h1", name="h1sb")
                            if s % 2 == 0:
                                nc.scalar.activation(h1sb[:], h1ps[:], AF.Relu,
                                                     bias=c1b[b][:, p:p + 1])
                            else:
                                nc.vector.tensor_scalar(h1sb[:], h1ps[:], c1b[b][:, p:p + 1],
                                                        0.0, ALU.add, ALU.max)
                            h2ps = pmlp.tile([E, 500], f32, tag="mm", name="h2ps")
                            nc.tensor.matmul(h2ps[:], w2d_sb[:], h1sb[:], start=True, stop=True)
                            h2sb = ma.tile([E, 500], f16, tag="h2", name="h2sb")
                            if s % 2 == 0:
                                nc.vector.tensor_scalar(h2sb[:], h2ps[:], b2b2_sb[:, 0:1],
                                                        0.0, ALU.add, ALU.max)
                            else:
                                nc.scalar.activation(h2sb[:], h2ps[:], AF.Relu,
                                                     bias=b2b2_sb[:, 0:1])
                            nc.tensor.matmul(s3ps[32 * s:32 * s + 32, :500], w3d_sb[:], h2sb[:],
                                             start=True, stop=True,
                                             tile_position=(0, 32 * s))
                        gs = ma.tile([E, 500], f32, tag="gs", name="gs")
                        nc.scalar.activation(gs[:], s3ps[:, :500], AF.Sigmoid, scale=-1.0,
                                             bias=nb3_sb[:, 0:1])
                        for s in range(4):
                            p = 4 * g + s
                            for a in range(2):
                                nc.sync.dma_start(
                                    Gt[b][p:p + 1, a * 500:(a + 1) * 500],
                                    gs[32 * s + a:32 * s + a + 1, :])

            # ============ PHASE 2: attention + pointer ============
            with (
                tc.tile_pool(name="ps_big", bufs=2, space="PSUM") as pbig,
                tc.tile_pool(name="ps_sum", bufs=1, space="PSUM") as psum_p,
                tc.tile_pool(name="ps_out", bufs=2, space="PSUM") as pout,
                tc.tile_pool(name="attact", bufs=3) as aa,
            ):
                for b in range(NB):
                    nin = aa.tile([P, N], f16, tag="nin", name="nin")
                    nc.sync.dma_start(nin[:], ninf[b])
                    nc.scalar.activation(enT[b][:], nin[:], AF.Exp)
                    G2 = aa.tile([P, 1024], f32, tag="G2", name="G2")
                    nc.vector.memset(G2[:, N:1024], 0.0)
                    nc.vector.tensor_mul(G2[:, :N], Gt[b][:], enT[b][:])
                    for half in range(2):
                        ps = pptr.tile([E, 512], f32, tag="ptr", name="ps")
                        for c in range(4):
                            t = half * 4 + c
                            nc.tensor.transpose(ps[:, c * P:(c + 1) * P],
                                                G2[:, t * 128:t * 128 + 128],
                                                identd[:P, :P])
                        nc.scalar.copy(GTt[b][:, half * 4 * P:(half + 1) * 4 * P],
                                       ps[:, :4 * P])

                    ocat = aa.tile([E, P], f32, tag="ocat", name="ocat")
                    for h in range(H):
                        q_ap = (qTt[b][h * D:(h + 1) * D, :] if h % 2 == 0
                                else qho[b][h // 2][:])
                        k_ap = (kTt[b] if h % 2 == 0 else kho[b][h // 2])
                        k_off = h * D if h % 2 == 0 else 0
                        egs = []
                        sums = psum_p.tile([1, 512], f32, tag="sums", name="sums")
                        ops = pout.tile([D, 512], f32, tag="ops", name="ops")
                        for half in range(2):
                            scps = pbig.tile([E, 4 * P], f32, tag="big", name="scps")
                            tp = (h * D if h % 2 == 0 else 0, 0)
                            for c in range(4):
                                t = half * 4 + c
                                nc.tensor.matmul(
                                    scps[:, c * P:(c + 1) * P],
                                    k_ap[k_off:k_off + D, t * 128:t * 128 + 128],
                                    q_ap, start=True, stop=True, tile_position=tp)
                            et = aa.tile([E, 4 * P], f32, tag="et", name="et")
                            nc.scalar.activation(et[:], scps[:], AF.Exp, scale=0.25)
                            eg = aa.tile([E, 4 * P], f32, tag="eg", name="eg")
                            nc.vector.tensor_mul(eg[:], et[:],
                                                 GTt[b][:, half * 4 * P:(half + 1) * 4 * P])
                            egs.append(eg)
                        for t in range(NT):
                            eg, c = egs[t // 4], t % 4
                            nc.tensor.matmul(sums[:1, :P], ones_sb[:, :],
                                             eg[:, c * P:(c + 1) * P],
                                             start=(t == 0), stop=(t == NT - 1))
                        for t in range(NT):
                            eg, c = egs[t // 4], t % 4
                            nc.tensor.matmul(ops[:D, :P],
                                             vsb[b][:, t * E + h * D:t * E + (h + 1) * D],
                                             eg[:, c * P:(c + 1) * P],
                                             start=(t == 0), stop=(t == NT - 1))
                        rs = aa.tile([1, P], f32, tag="rs", name="rs")
                        nc.vector.reciprocal(rs[:], sums[:1, :P])
                        rb = aa.tile([D, P], f32, tag="rb", name="rb")
                        nc.gpsimd.partition_broadcast(rb[:], rs[:])
                        oh = aa.tile([D, P], f32, tag="oh", name="oh")
                        nc.vector.tensor_mul(oh[:], ops[:D, :P], rb[:])
                        nc.sync.dma_start(ocat[h * D:(h + 1) * D, :], oh[:])

                    ps = pptr.tile([E, 512], f32, tag="ptr", name="ps")
                    nc.tensor.matmul(ps[:, :P], wcomb_sb[:], ocat[:], start=True, stop=True)
                    nc.scalar.activation(mhT[b][:], ps[:, :P], AF.Identity,
                                         bias=bcomb_sb[:, 0:1])

                    us = []
                    acc = [aa.tile([P, 1], f32, tag=f"acc{i}", name=f"acc{i}")
                           for i in range(2)]
                    for half in range(2):
                        ptp = pbig.tile([P, 500], f32, tag="big", name="ptp")
                        nc.tensor.matmul(ptp[:], mhT[b][:],
                                         ndT[b][:, half * 500:(half + 1) * 500],
                                         start=True, stop=True)
                        th = aa.tile([P, 500], f32, tag="th", name="th")
                        nc.scalar.activation(th[:], ptp[:], AF.Tanh,
                                             scale=float(1.0 / np.sqrt(E)))
                        e2 = aa.tile([P, 500], f32, tag="e2", name="e2")
                        nc.scalar.activation(e2[:], th[:], AF.Exp, scale=float(CLIP))
                        u = aa.tile([P, 500], f32, tag=f"u{half}", name=f"u{half}")
                        nc.vector.tensor_mul(u[:], e2[:],
                                             enT[b][:, half * 500:(half + 1) * 500])
                        nc.vector.tensor_reduce(acc[half][:, 0:1], u[:],
                                                mybir.AxisListType.X, ALU.add)
                        us.append(u)
                    tot = aa.tile([P, 1], f32, tag="tot", name="tot")
                    nc.vector.tensor_add(tot[:], acc[0][:], acc[1][:])
                    rp = aa.tile([P, 1], f32, tag="rp", name="rp")
                    nc.vector.reciprocal(rp[:], tot[:])
                    for half in range(2):
                        pr = aa.tile([P, 500], bf16, tag="pr", name="pr")
                        nc.vector.tensor_scalar(pr[:], us[half][:], rp[:, 0:1], None, ALU.mult)
                        nc.sync.dma_start(probs[b, :, half * 500:(half + 1) * 500], pr[:])

    nc.compile()
    return nc


_STATE = None

# weight blobs: (name, per-core shape); f16 set and f32 set
WSPEC16 = [("wl1t", (4, 24, E)), ("w2d", (E, E)), ("w3d", (E, 32)),
           ("wq2", (E, E)), ("fw1a", (E, E)), ("fw2a", (E, E)),
           ("wk", (E, E)), ("wv", (E, E)), ("wcomb", (E, E)),
           ("identm", (E, E))]
WSPEC32 = [("wq2b", (1, E)), ("fw1b", (1, E)), ("fw1c", (E, 1)),
           ("fw1d", (1, 1)), ("fw2b", (1, E)), ("fb1", (E, 1)),
           ("fb1e", (1, 1)), ("fb2", (E, 1)), ("lqw", (E, QDIM)),
           ("lqwb", (1, QDIM)), ("lqb", (QDIM, 1)), ("lmw1q", (QDIM, HID)),
           ("lmb1", (HID, 1)), ("b2b2", (E, 1)), ("nb3", (E, 1)),
           ("bcomb", (E, 1))]
N16 = sum(int(np.prod(s)) for _, s in WSPEC16)
N32 = sum(int(np.prod(s)) for _, s in WSPEC32)


def _get_state():
    global _STATE
    if _STATE is not None:
        return _STATE
    nc = _build_nc()
    install_neuronx_cc_hook()

    partition_name = (nc.partition_id_tensor.name
                      if nc.partition_id_tensor else None)
    in_names, out_names, out_avals = [], [], []
    for alloc in nc.m.functions[0].allocations:
        if not isinstance(alloc, mybir.MemoryLocationSet):
            continue
        name = alloc.memorylocations[0].name
        if alloc.kind == "ExternalInput":
            if name != partition_name:
                in_names.append(name)
        elif alloc.kind == "ExternalOutput":
            out_names.append(name)
            out_avals.append(jax.core.ShapedArray(
                tuple(alloc.tensor_shape), mybir.dt.np(alloc.dtype)))
    n_params = len(in_names)
    all_names = in_names + out_names + ([partition_name] if partition_name else [])
    donate = tuple(range(n_params, n_params + len(out_names)))

    def _body(*args):
        operands = list(args)
        if partition_name is not None:
            operands.append(bass2jax.partition_id_tensor())
        outs = _bass_exec_p.bind(
            *operands, out_avals=tuple(out_avals), in_names=tuple(all_names),
            out_names=tuple(out_names), lowering_input_output_aliases=(),
            sim_require_finite=True, sim_require_nnan=True, nc=nc)
        return tuple(outs)

    devices = jax.devices()[:NCORES]
    mesh = Mesh(np.asarray(devices), ("core",))
    in_specs = (PartitionSpec("core"),) * (n_params + len(out_names))
    out_specs = (PartitionSpec("core"),) * len(out_names)
    sharded = jax.jit(
        shard_map(_body, mesh=mesh, in_specs=in_specs, out_specs=out_specs,
                  check_rep=False),
        donate_argnums=donate, keep_unused=True)
    shard = NamedSharding(mesh, PartitionSpec("core"))
    zeros_fn = jax.jit(
        lambda: jnp.zeros((NCORES * NB, P, N), jnp.bfloat16),
        out_shardings=shard)
    # device-resident f16 zeros reused whenever ninf_mask is all-zero
    ninf_zero = jax.jit(lambda: jnp.zeros((NCORES * NB, P, N), jnp.float16),
                        out_shardings=shard)()
    jax.block_until_ready(ninf_zero)

    # weights travel as two small blobs to core 0 only (cores 1-7 get
    # device-made zeros); an on-device psum broadcasts, then local slices
    # rebuild each weight in the tiled P("core") layout the NEFF expects
    def _wunpack_body(b16, b32):
        b16 = jax.lax.psum(b16, "core")
        b32 = jax.lax.psum(b32, "core")
        outs = []
        for blob, spec in ((b16, WSPEC16), (b32, WSPEC32)):
            off = 0
            for _, shp in spec:
                n = int(np.prod(shp))
                outs.append(blob[:, off:off + n].reshape((1,) + shp))
                off += n
        return tuple(outs)

    nsp = len(WSPEC16) + len(WSPEC32)
    wunpack = jax.jit(shard_map(
        _wunpack_body, mesh=mesh,
        in_specs=(PartitionSpec("core"),) * 2,
        out_specs=(PartitionSpec("core"),) * nsp, check_rep=False))
    wz16 = jax.jit(lambda: jnp.zeros((NCORES, N16), jnp.float16),
                   out_shardings=shard)()
    wz32 = jax.jit(lambda: jnp.zeros((NCORES, N32), jnp.float32),
                   out_shardings=shard)()
    jax.block_until_ready([wz16, wz32])
    z16, z32 = [None] * NCORES, [None] * NCORES
    for s in wz16.addressable_shards:
        z16[devices.index(s.device)] = s.data
    for s in wz32.addressable_shards:
        z32[devices.index(s.device)] = s.data

    _STATE = {"nc": nc, "sharded": sharded, "in_names": in_names,
              "shard": shard, "zeros_fn": zeros_fn,
              "donate_bufs": [None] * CH, "ninf_zero": ninf_zero,
              "wunpack": wunpack, "z16": z16, "z32": z32,
              "devices": devices, "wnames": [n for n, _ in WSPEC16 + WSPEC32]}
    return _STATE


def _prep_weights(inp):
    w = {}
    lm_W1 = inp["lm_W1"]
    wl1t = np.zeros((4, 24, E), np.float32)
    for s in range(4):
        for a in range(2):
            for c in range(DYN):
                wl1t[s, 6 * s + 3 * a + c, 64 * a:64 * a + HID] = lm_W1[c]
    w["wl1t"] = wl1t
    w2d = np.zeros((E, E), np.float32)
    w2d[:HID, :HID] = inp["lm_W2"]
    w2d[HID:, HID:] = inp["lm_W2"]
    w["w2d"] = w2d
    w3d = np.zeros((E, 32), np.float32)
    w3d[:HID, 0] = inp["lm_W3"][:, 0]
    w3d[HID:, 1] = inp["lm_W3"][:, 0]
    w["w3d"] = w3d
    w["wq2"] = 2.0 * inp["Wq_last"][:E]
    w["wq2b"] = 2.0 * inp["Wq_last"][E:E + 1]
    w["fw1a"] = inp["film_W1"][:E, :E]
    w["fw1b"] = inp["film_W1"][E:E + 1, :E]
    w["fw1c"] = inp["film_W1"][:E, E:E + 1]
    w["fw1d"] = inp["film_W1"][E:E + 1, E:E + 1]
    w["fw2a"] = inp["film_W2"][:E]
    w["fw2b"] = inp["film_W2"][E:E + 1]
    w["fb1"] = inp["film_b1"][:E, None]
    w["fb1e"] = inp["film_b1"][E:E + 1, None]
    w["fb2"] = inp["film_b2"][:, None]
    w["lqw"] = inp["lazy_q_W"][:E]
    w["lqwb"] = inp["lazy_q_W"][E:E + 1]
    w["lqb"] = inp["lazy_q_b"][:, None]
    w["lmw1q"] = lm_W1[DYN:]
    w["lmb1"] = inp["lm_b1"][:, None]
    w["b2b2"] = np.concatenate([inp["lm_b2"], inp["lm_b2"]])[:, None]
    w["nb3"] = np.full((E, 1), -float(inp["lm_b3"][0]), np.float32)
    w["wk"] = inp["Wk"]
    w["wv"] = inp["Wv"]
    w["wcomb"] = inp["W_comb"]
    w["bcomb"] = inp["b_comb"][:, None]
    w["identm"] = np.eye(E, dtype=np.float32)
    # pack into one f16 blob + one f32 blob (per-core layout)
    b16 = np.empty((1, N16), np.float16)
    b32 = np.empty((1, N32), np.float32)
    for blob, spec in ((b16, WSPEC16), (b32, WSPEC32)):
        off = 0
        for k, shp in spec:
            n = int(np.prod(shp))
            blob[0, off:off + n] = np.asarray(w[k], dtype=blob.dtype).ravel()
            off += n
    return b16, b32


# lazy-MLP input layout: (P,N,3) -> 25 groups of 4 pomo rows, each row's
# 1000 nodes split in 2 halves of 500, features transposed to rows
def _xt_chunk(dyn, lo, hi):
    return np.ascontiguousarray(
        dyn[lo:hi].reshape(hi - lo, 25, 4, 2, 500, 3).transpose(0, 1, 2, 3, 5, 4)
    ).reshape(hi - lo, 25, 24, 500).astype(np.float16)


def kernel(**inputs):
    st = _get_state()
    inp = {k: np.asarray(v, dtype=np.float32) for k, v in inputs.items()}
    shard = st["shard"]
    devices = st["devices"]
    bpch = NCORES * NB              # batches per chunk

    # weights: small blobs to core 0, on-device broadcast + unpack
    b16, b32 = _prep_weights(inp)
    d16 = jax.device_put(b16, devices[0])
    d32 = jax.device_put(b32, devices[0])
    g16 = jax.make_array_from_single_device_arrays(
        (NCORES, N16), shard, [d16] + st["z16"][1:])
    g32 = jax.make_array_from_single_device_arrays(
        (NCORES, N32), shard, [d32] + st["z32"][1:])
    wdev = dict(zip(st["wnames"], st["wunpack"](g16, g32)))

    nodes_f16 = inp["encoded_nodes"].astype(np.float16)
    last_f16 = inp["encoded_last_node"].astype(np.float16)
    loadv = np.ascontiguousarray(inp["load"][:, None, :])
    ninf32 = inp["ninf_mask"]
    ninf_any = bool(ninf32.any())
    dyn = inp["dyn_features"]

    # chunk 0 is fully uploaded + dispatched before chunk 1's upload, so
    # chunk 0's exec + output fetch overlap chunk 1's transfers
    outs = []
    for ci in range(CH):
        lo, hi = ci * bpch, (ci + 1) * bpch
        g = {"nodes": jax.device_put(nodes_f16[lo:hi], shard),
             "last": jax.device_put(last_f16[lo:hi], shard),
             "loadv": jax.device_put(loadv[lo:hi], shard)}
        if ninf_any:
            g["ninf"] = jax.device_put(ninf32[lo:hi].astype(np.float16), shard)
        else:
            g["ninf"] = st["ninf_zero"]
        g["xt"] = _xt_chunk(dyn, lo, hi)
        g.update(wdev)
        args = [g[n] for n in st["in_names"]]
        donate_buf = st["donate_bufs"][ci]
        if donate_buf is None:
            donate_buf = st["zeros_fn"]()
        (o,) = st["sharded"](*args, donate_buf)
        st["donate_bufs"][ci] = o       # donated into the next call
        o.copy_to_host_async()
        outs.append(o)
    res = np.empty((B, P, N), np.float32)
    for ci, o in enumerate(outs):       # chunk 0 converts while chunk 1 runs
        res[ci * bpch:(ci + 1) * bpch] = np.asarray(o)
    return res


# revision 16
# speedup vs baseline: 1.1052x; 1.1052x over previous
import numpy as np
import jax
import jax.numpy as jnp
from jax.sharding import Mesh, PartitionSpec, NamedSharding
from jax.experimental.shard_map import shard_map

import concourse.tile as tile
from concourse import bacc, mybir
from concourse import bass2jax
from concourse.bass2jax import _bass_exec_p, install_neuronx_cc_hook

E, H, D = 128, 8, 16
QDIM, DYN, HID = 16, 3, 64
CLIP = 10.0
NCORES = 8
B, P, N = 16, 100, 1000
NB = 1                     # batches per core per launch
CH = B // (NCORES * NB)    # launches (chunks) per call
NT = (N + 127) // 128      # n tiles of 128 (last = 104)
LASTN = N - 128 * (NT - 1)

f32 = mybir.dt.float32
f16 = mybir.dt.float16
bf16 = mybir.dt.bfloat16
AF = mybir.ActivationFunctionType
ALU = mybir.AluOpType

def _build_nc():
    nc = bacc.Bacc("TRN2", target_bir_lowering=False, debug=False,
                   num_devices=NCORES)

    def din(name, shape, dt=f32):
        return nc.dram_tensor(name, shape, dt, kind="ExternalInput").ap()

    xt = din("xt", [NB, 25, 24, 500], f16)
    nodes = din("nodes", [NB, N, E], f16)
    last = din("last", [NB, P, E], f16)
    loadv = din("loadv", [NB, 1, P])
    ninf = din("ninf", [NB, P, N], f16)
    wl1t = din("wl1t", [4, 24, E], f16)   # L1 templates (dyn rows only)
    w2d = din("w2d", [E, E], f16)
    w3d = din("w3d", [E, 32], f16)
    wq2 = din("wq2", [E, E], f16)
    wq2b = din("wq2b", [1, E])
    fw1a = din("fw1a", [E, E], f16)
    fw1b = din("fw1b", [1, E])
    fw1c = din("fw1c", [E, 1])
    fw1d = din("fw1d", [1, 1])
    fw2a = din("fw2a", [E, E], f16)
    fw2b = din("fw2b", [1, E])
    fb1 = din("fb1", [E, 1])
    fb1e = din("fb1e", [1, 1])
    fb2 = din("fb2", [E, 1])
    lqw = din("lqw", [E, QDIM])
    lqwb = din("lqwb", [1, QDIM])
    lqb = din("lqb", [QDIM, 1])
    lmw1q = din("lmw1q", [QDIM, HID])
    lmb1 = din("lmb1", [HID, 1])
    b2b2 = din("b2b2", [E, 1])
    nb3 = din("nb3", [E, 1])
    wk = din("wk", [E, E], f16)
    wv = din("wv", [E, E], f16)
    wcomb = din("wcomb", [E, E], f16)
    bcomb = din("bcomb", [E, 1])
    identm = din("identm", [E, E], f16)
    probs = nc.dram_tensor("probs", [NB, P, N], bf16, kind="ExternalOutput").ap()

    with tile.TileContext(nc) as tc:
        with (
            tc.tile_pool(name="wpool", bufs=1) as wp,
            tc.tile_pool(name="perb", bufs=1) as pb,
            tc.tile_pool(name="sball", bufs=3) as sb,
            tc.tile_pool(name="ps_ptr", bufs=2, space="PSUM") as pptr,
        ):
            def wtile(shape, dt, src, tag):
                t = wp.tile(shape, dt, tag=tag, name=tag)
                nc.sync.dma_start(t[:], src)
                return t

            identh = wtile([E, E], f16, identm, "identh")
            identd = wp.tile([E, E], f32, tag="identd", name="identd")
            nc.scalar.copy(identd[:], identh[:])
            w2d_sb = wtile([E, E], f16, w2d, "w2d")
            w3d_sb = wtile([E, 32], f16, w3d, "w3d")
            w1l = [wtile([24, E], f16, wl1t[s], f"w1l{s}") for s in range(4)]
            # big f32 weights arrive as f16, upconvert once on device
            w16sb = {}
            for nm, src in (("wq2", wq2), ("fw1a", fw1a), ("fw2a", fw2a),
                            ("wk", wk), ("wv", wv), ("wcomb", wcomb)):
                t16 = wtile([E, E], f16, src, nm + "_16")
                t32 = wp.tile([E, E], f32, tag=nm, name=nm)
                nc.scalar.copy(t32[:], t16[:])
                w16sb[nm] = t32
            wq2_sb = w16sb["wq2"]
            fw1a_sb = w16sb["fw1a"]
            fw2a_sb = w16sb["fw2a"]
            wk_sb = w16sb["wk"]
            wv_sb = w16sb["wv"]
            wcomb_sb = w16sb["wcomb"]
            wq2b_sb = wtile([1, E], f32, wq2b, "wq2b")
            fw1b_sb = wtile([1, E], f32, fw1b, "fw1b")
            fw1c_sb = wtile([E, 1], f32, fw1c, "fw1c")
            fw1d_sb = wtile([1, 1], f32, fw1d, "fw1d")
            fw2b_sb = wtile([1, E], f32, fw2b, "fw2b")
            fb1_sb = wtile([E, 1], f32, fb1, "fb1")
            fb1e_sb = wtile([1, 1], f32, fb1e, "fb1e")
            fb2_sb = wtile([E, 1], f32, fb2, "fb2")
            lqw_sb = wtile([E, QDIM], f32, lqw, "lqw")
            lqwb_sb = wtile([1, QDIM], f32, lqwb, "lqwb")
            lqb_sb = wtile([QDIM, 1], f32, lqb, "lqb")
            lmw1q_sb = wtile([QDIM, HID], f32, lmw1q, "lmw1q")
            lmb1_sb = wtile([HID, 1], f32, lmb1, "lmb1")
            b2b2_sb = wtile([E, 1], f32, b2b2, "b2b2")
            nb3_sb = wtile([E, 1], f32, nb3, "nb3")
            bcomb_sb = wtile([E, 1], f32, bcomb, "bcomb")
            ones_sb = wp.tile([E, 1], f32, tag="ones", name="ones")
            nc.vector.memset(ones_sb[:], 1.0)

            ndT = [pb.tile([E, 1024], f32, tag=f"ndT{b}", name=f"ndT{b}") for b in range(NB)]
            kTt = [pb.tile([E, 1024], f32, tag=f"kT{b}", name=f"kT{b}") for b in range(NB)]
            vsb = [pb.tile([E, NT * E], f32, tag=f"v{b}", name=f"v{b}") for b in range(NB)]
            qTt = [pb.tile([E, P], f32, tag=f"qT{b}", name=f"qT{b}") for b in range(NB)]
            qho = [[pb.tile([D, P], f32, tag=f"qh{b}{h}", name=f"qh{b}{h}")
                    for h in range(1, H, 2)] for b in range(NB)]
            kho = [[pb.tile([D, 1024], f32, tag=f"kh{b}{h}", name=f"kh{b}{h}")
                    for h in range(1, H, 2)] for b in range(NB)]
            Gt = [pb.tile([P, N], f32, tag=f"G{b}", name=f"G{b}") for b in range(NB)]
            GTt = [pb.tile([E, NT * P], f32, tag=f"GT{b}", name=f"GT{b}") for b in range(NB)]
            mhT = [pb.tile([E, P], f32, tag=f"mh{b}", name=f"mh{b}") for b in range(NB)]
            enT = [pb.tile([P, N], f32, tag=f"en{b}", name=f"en{b}") for b in range(NB)]
            c1b = [pb.tile([E, P], f32, tag=f"c1b{b}", name=f"c1b{b}") for b in range(NB)]

            # ============ PHASE 1: prep + lazy-mask MLP ============
            with (
                tc.tile_pool(name="ps_mlp", bufs=3, space="PSUM") as pmlp,
                tc.tile_pool(name="ps_s3", bufs=2, space="PSUM") as ps3,
                tc.tile_pool(name="xtp", bufs=3) as xtp,
                tc.tile_pool(name="mlpact", bufs=3) as ma,
            ):
                for b in range(NB):
                    nc.vector.memset(ndT[b][:, N:1024], 0.0)
                    nc.vector.memset(kTt[b][:, N:1024], 0.0)
                    # nodes^T
                    for half in range(2):
                        ps = pptr.tile([E, 512], f16, tag="ptr", name="ps")
                        for c in range(4):
                            t = half * 4 + c
                            rows = LASTN if t == NT - 1 else 128
                            nd_in = ma.tile([128, E], f16, tag="ndin", name="nd_in")
                            nc.sync.dma_start(nd_in[:rows, :],
                                              nodes[b, t * 128:t * 128 + rows, :])
                            nc.tensor.transpose(ps[:, c * 128:c * 128 + rows],
                                                nd_in[:rows, :], identh[:rows, :rows])
                        w = 512 if half == 0 else N - 512
                        nc.scalar.copy(ndT[b][:, half * 512:half * 512 + w], ps[:, :w])
                    # kT
                    for half in range(2):
                        ps = pptr.tile([E, 512], f32, tag="ptr", name="ps")
                        nc.tensor.matmul(ps[:, :500], wk_sb[:],
                                         ndT[b][:, half * 500:half * 500 + 500],
                                         start=True, stop=True)
                        nc.scalar.copy(kTt[b][:, half * 500:half * 500 + 500], ps[:, :500])
                    for i, h in enumerate(range(1, H, 2)):
                        nc.sync.dma_start(kho[b][i][:], kTt[b][h * D:(h + 1) * D, :])
                    # v natural
                    for half in range(2):
                        ps = pptr.tile([E, 512], f32, tag="ptr", name="ps")
                        for c in range(4):
                            t = half * 4 + c
                            nc.tensor.matmul(ps[:, c * 128:c * 128 + 128],
                                             ndT[b][:, t * 128:t * 128 + 128],
                                             wv_sb[:], start=True, stop=True)
                        nc.scalar.copy(vsb[b][:, half * 512:half * 512 + 512], ps[:])
                    # q path
                    la_in = ma.tile([P, E], f16, tag="lain", name="la_in")
                    nc.sync.dma_start(la_in[:], last[b])
                    lo_in = ma.tile([1, P], f32, tag="loin", name="lo_in")
                    nc.sync.dma_start(lo_in[:], loadv[b])
                    ps = pptr.tile([E, 512], f16, tag="ptr", name="ps")
                    nc.tensor.transpose(ps[:, :P], la_in[:], identh[:P, :P])
                    laT = ma.tile([E, P], f32, tag="laT", name="laT")
                    nc.scalar.copy(laT[:], ps[:, :P])

                    ps = pptr.tile([E, 512], f32, tag="ptr", name="ps")
                    nc.tensor.matmul(ps[:, :P], fw1a_sb[:], laT[:], start=True, stop=False)
                    nc.tensor.matmul(ps[:, :P], fw1b_sb[:], lo_in[:], start=False, stop=True)
                    r128 = ma.tile([E, P], f32, tag="r128", name="r128")
                    nc.scalar.activation(r128[:], ps[:, :P], AF.Relu, bias=fb1_sb[:, 0:1])
                    ps = pptr.tile([E, 512], f32, tag="ptr", name="ps")
                    nc.tensor.matmul(ps[:1, :P], fw1c_sb[:], laT[:], start=True, stop=False)
                    nc.tensor.matmul(ps[:1, :P], fw1d_sb[:], lo_in[:], start=False, stop=True)
                    rl = ma.tile([1, P], f32, tag="rl", name="rl")
                    nc.scalar.activation(rl[:], ps[:1, :P], AF.Relu, bias=fb1e_sb[:, 0:1])

                    ps = pptr.tile([E, 512], f32, tag="ptr", name="ps")
                    nc.tensor.matmul(ps[:, :P], fw2a_sb[:], r128[:], start=True, stop=False)
                    nc.tensor.matmul(ps[:, :P], fw2b_sb[:], rl[:], start=False, stop=True)
                    sig = ma.tile([E, P], f32, tag="sig", name="sig")
                    nc.scalar.activation(sig[:], ps[:, :P], AF.Sigmoid, bias=fb2_sb[:, 0:1])

                    ps = pptr.tile([E, 512], f32, tag="ptr", name="ps")
                    nc.tensor.matmul(ps[:, :P], wq2_sb[:], laT[:], start=True, stop=False)
                    nc.tensor.matmul(ps[:, :P], wq2b_sb[:], lo_in[:], start=False, stop=True)
                    qraw = ma.tile([E, P], f32, tag="qraw", name="qraw")
                    nc.scalar.copy(qraw[:], ps[:, :P])
                    nc.vector.tensor_mul(qTt[b][:], qraw[:], sig[:])
                    for i, h in enumerate(range(1, H, 2)):
                        nc.sync.dma_start(qho[b][i][:], qTt[b][h * D:(h + 1) * D, :])

                    ps = pptr.tile([E, 512], f32, tag="ptr", name="ps")
                    nc.tensor.matmul(ps[:QDIM, :P], lqw_sb[:], laT[:], start=True, stop=False)
                    nc.tensor.matmul(ps[:QDIM, :P], lqwb_sb[:], lo_in[:], start=False, stop=True)
                    qfT = ma.tile([QDIM, P], f32, tag="qfT", name="qfT")
                    nc.scalar.activation(qfT[:], ps[:QDIM, :P], AF.Identity, bias=lqb_sb[:, 0:1])
                    ps = pptr.tile([E, 512], f32, tag="ptr", name="ps")
                    nc.tensor.matmul(ps[:HID, :P], lmw1q_sb[:], qfT[:], start=True, stop=True)
                    c1s = ma.tile([HID, P], f32, tag="c1s", name="c1s")
                    nc.scalar.activation(c1s[:], ps[:HID, :P], AF.Identity, bias=lmb1_sb[:, 0:1])
                    nc.sync.dma_start(c1b[b][0:HID, :], c1s[:])
                    nc.sync.dma_start(c1b[b][HID:E, :], c1s[:])

                    # lazy MLP
                    for g in range(25):
                        xg = xtp.tile([24, 500], f16, tag="xg", name="xg")
                        nc.sync.dma_start(xg[:], xt[b, g])
                        s3ps = ps3.tile([E, 512], f32, tag="s3", name="s3ps")
                        for s in range(4):
                            p = 4 * g + s
                            h1ps = pmlp.tile([E, 500], f32, tag="mm", name="h1ps")
                            nc.tensor.matmul(h1ps[:], w1l[s][:], xg[:], start=True, stop=True)
                            h1sb = ma.tile([E, 500], f16, tag="h1", name="h1sb")
                            if s % 2 == 0:
                                nc.scalar.activation(h1sb[:], h1ps[:], AF.Relu,
                                                     bias=c1b[b][:, p:p + 1])
                            else:
                                nc.vector.tensor_scalar(h1sb[:], h1ps[:], c1b[b][:, p:p + 1],
                                                        0.0, ALU.add, ALU.max)
                            h2ps = pmlp.tile([E, 500], f32, tag="mm", name="h2ps")
                            nc.tensor.matmul(h2ps[:], w2d_sb[:], h1sb[:], start=True, stop=True)
                            h2sb = ma.tile([E, 500], f16, tag="h2", name="h2sb")
                            if s % 2 == 0:
                                nc.vector.tensor_scalar(h2sb[:], h2ps[:], b2b2_sb[:, 0:1],
                                                        0.0, ALU.add, ALU.max)
                            else:
                                nc.scalar.activation(h2sb[:], h2ps[:], AF.Relu,
                                                     bias=b2b2_sb[:, 0:1])
                            nc.tensor.matmul(s3ps[32 * s:32 * s + 32, :500], w3d_sb[:], h2sb[:],
                                             start=True, stop=True,
                                             tile_position=(0, 32 * s))
                        gs = ma.tile([E, 500], f32, tag="gs", name="gs")
                        nc.scalar.activation(gs[:], s3ps[:, :500], AF.Sigmoid, scale=-1.0,
                                             bias=nb3_sb[:, 0:1])
                        for s in range(4):
                            p = 4 * g + s
                            for a in range(2):
                                nc.sync.dma_start(
                                    Gt[b][p:p + 1, a * 500:(a + 1) * 500],
                                    gs[32 * s + a:32 * s + a + 1, :])

            # ============ PHASE 2: attention + pointer ============
            with (
                tc.tile_pool(name="ps_big", bufs=2, space="PSUM") as pbig,
                tc.tile_pool(name="ps_sum", bufs=1, space="PSUM") as psum_p,
                tc.tile_pool(name="ps_out", bufs=2, space="PSUM") as pout,
                tc.tile_pool(name="attact", bufs=3) as aa,
            ):
                for b in range(NB):
                    nin = aa.tile([P, N], f16, tag="nin", name="nin")
                    nc.sync.dma_start(nin[:], ninf[b])
                    nc.scalar.activation(enT[b][:], nin[:], AF.Exp)
                    G2 = aa.tile([P, 1024], f32, tag="G2", name="G2")
                    nc.vector.memset(G2[:, N:1024], 0.0)
                    nc.vector.tensor_mul(G2[:, :N], Gt[b][:], enT[b][:])
                    for half in range(2):
                        ps = pptr.tile([E, 512], f32, tag="ptr", name="ps")
                        for c in range(4):
                            t = half * 4 + c
                            nc.tensor.transpose(ps[:, c * P:(c + 1) * P],
                                                G2[:, t * 128:t * 128 + 128],
                                                identd[:P, :P])
                        nc.scalar.copy(GTt[b][:, half * 4 * P:(half + 1) * 4 * P],
                                       ps[:, :4 * P])

                    ocat = aa.tile([E, P], f32, tag="ocat", name="ocat")
                    for h in range(H):
                        q_ap = (qTt[b][h * D:(h + 1) * D, :] if h % 2 == 0
                                else qho[b][h // 2][:])
                        k_ap = (kTt[b] if h % 2 == 0 else kho[b][h // 2])
                        k_off = h * D if h % 2 == 0 else 0
                        egs = []
                        sums = psum_p.tile([1, 512], f32, tag="sums", name="sums")
                        ops = pout.tile([D, 512], f32, tag="ops", name="ops")
                        for half in range(2):
                            scps = pbig.tile([E, 4 * P], f32, tag="big", name="scps")
                            tp = (h * D if h % 2 == 0 else 0, 0)
                            for c in range(4):
                                t = half * 4 + c
                                nc.tensor.matmul(
                                    scps[:, c * P:(c + 1) * P],
                                    k_ap[k_off:k_off + D, t * 128:t * 128 + 128],
                                    q_ap, start=True, stop=True, tile_position=tp)
                            et = aa.tile([E, 4 * P], f32, tag="et", name="et")
                            nc.scalar.activation(et[:], scps[:], AF.Exp, scale=0.25)
                            eg = aa.tile([E, 4 * P], f32, tag="eg", name="eg")
                            nc.vector.tensor_mul(eg[:], et[:],
                                                 GTt[b][:, half * 4 * P:(half + 1) * 4 * P])
                            egs.append(eg)
                        for t in range(NT):
                            eg, c = egs[t // 4], t % 4
                            nc.tensor.matmul(sums[:1, :P], ones_sb[:, :],
                                             eg[:, c * P:(c + 1) * P],
                                             start=(t == 0), stop=(t == NT - 1))
                        for t in range(NT):
                            eg, c = egs[t // 4], t % 4
                            nc.tensor.matmul(ops[:D, :P],
                                             vsb[b][:, t * E + h * D:t * E + (h + 1) * D],
                                             eg[:, c * P:(c + 1) * P],
                                             start=(t == 0), stop=(t == NT - 1))
                        rs = aa.tile([1, P], f32, tag="rs", name="rs")
                        nc.vector.reciprocal(rs[:], sums[:1, :P])
                        rb = aa.tile([D, P], f32, tag="rb", name="rb")
                        nc.gpsimd.partition_broadcast(rb[:], rs[:])
                        oh = aa.tile([D, P], f32, tag="oh", name="oh")
                        nc.vector.tensor_mul(oh[:], ops[:D, :P], rb[:])
                        nc.sync.dma_start(ocat[h * D:(h + 1) * D, :], oh[:])

                    ps = pptr.tile([E, 512], f32, tag="ptr", name="ps")
                    nc.tensor.matmul(ps[:, :P], wcomb_sb[:], ocat[:], start=True, stop=True)
                    nc.scalar.activation(mhT[b][:], ps[:, :P], AF.Identity,
                                         bias=bcomb_sb[:, 0:1])

                    us = []
                    acc = [aa.tile([P, 1], f32, tag=f"acc{i}", name=f"acc{i}")
                           for i in range(2)]
                    for half in range(2):
                        ptp = pbig.tile([P, 500], f32, tag="big", name="ptp")
                        nc.tensor.matmul(ptp[:], mhT[b][:],
                                         ndT[b][:, half * 500:(half + 1) * 500],
                                         start=True, stop=True)
                        th = aa.tile([P, 500], f32, tag="th", name="th")
                        nc.scalar.activation(th[:], ptp[:], AF.Tanh,
                                             scale=float(1.0 / np.sqrt(E)))
                        e2 = aa.tile([P, 500], f32, tag="e2", name="e2")
                        nc.scalar.activation(e2[:], th[:], AF.Exp, scale=float(CLIP))
                        u = aa.tile([P, 500], f32, tag=f"u{half}", name=f"u{half}")
                        nc.vector.tensor_mul(u[:], e2[:],
                                             enT[b][:, half * 500:(half + 1) * 500])
                        nc.vector.tensor_reduce(acc[half][:, 0:1], u[:],
                                                mybir.AxisListType.X, ALU.add)
                        us.append(u)
                    tot = aa.tile([P, 1], f32, tag="tot", name="tot")
                    nc.vector.tensor_add(tot[:], acc[0][:], acc[1][:])
                    rp = aa.tile([P, 1], f32, tag="rp", name="rp")
                    nc.vector.reciprocal(rp[:], tot[:])
                    for half in range(2):
                        pr = aa.tile([P, 500], bf16, tag="pr", name="pr")
                        nc.vector.tensor_scalar(pr[:], us[half][:], rp[:, 0:1], None, ALU.mult)
                        nc.sync.dma_start(probs[b, :, half * 500:(half + 1) * 500], pr[:])

    nc.compile()
    return nc


_STATE = None

# weight blobs: (name, per-core shape); f16 set and f32 set
WSPEC16 = [("wl1t", (4, 24, E)), ("w2d", (E, E)), ("w3d", (E, 32)),
           ("wq2", (E, E)), ("fw1a", (E, E)), ("fw2a", (E, E)),
           ("wk", (E, E)), ("wv", (E, E)), ("wcomb", (E, E)),
           ("identm", (E, E))]
WSPEC32 = [("wq2b", (1, E)), ("fw1b", (1, E)), ("fw1c", (E, 1)),
           ("fw1d", (1, 1)), ("fw2b", (1, E)), ("fb1", (E, 1)),
           ("fb1e", (1, 1)), ("fb2", (E, 1)), ("lqw", (E, QDIM)),
           ("lqwb", (1, QDIM)), ("lqb", (QDIM, 1)), ("lmw1q", (QDIM, HID)),
           ("lmb1", (HID, 1)), ("b2b2", (E, 1)), ("nb3", (E, 1)),
           ("bcomb", (E, 1))]
N16 = sum(int(np.prod(s)) for _, s in WSPEC16)
N32 = sum(int(np.prod(s)) for _, s in WSPEC32)


def _get_state():
    global _STATE
    if _STATE is not None:
        return _STATE
    nc = _build_nc()
    install_neuronx_cc_hook()

    partition_name = (nc.partition_id_tensor.name
                      if nc.partition_id_tensor else None)
    in_names, out_names, out_avals = [], [], []
    for alloc in nc.m.functions[0].allocations:
        if not isinstance(alloc, mybir.MemoryLocationSet):
            continue
        name = alloc.memorylocations[0].name
        if alloc.kind == "ExternalInput":
            if name != partition_name:
                in_names.append(name)
        elif alloc.kind == "ExternalOutput":
            out_names.append(name)
            out_avals.append(jax.core.ShapedArray(
                tuple(alloc.tensor_shape), mybir.dt.np(alloc.dtype)))
    n_params = len(in_names)
    all_names = in_names + out_names + ([partition_name] if partition_name else [])
    donate = tuple(range(n_params, n_params + len(out_names)))

    def _body(*args):
        operands = list(args)
        if partition_name is not None:
            operands.append(bass2jax.partition_id_tensor())
        outs = _bass_exec_p.bind(
            *operands, out_avals=tuple(out_avals), in_names=tuple(all_names),
            out_names=tuple(out_names), lowering_input_output_aliases=(),
            sim_require_finite=True, sim_require_nnan=True, nc=nc)
        return tuple(outs)

    devices = jax.devices()[:NCORES]
    mesh = Mesh(np.asarray(devices), ("core",))
    in_specs = (PartitionSpec("core"),) * (n_params + len(out_names))
    out_specs = (PartitionSpec("core"),) * len(out_names)
    sharded = jax.jit(
        shard_map(_body, mesh=mesh, in_specs=in_specs, out_specs=out_specs,
                  check_rep=False),
        donate_argnums=donate, keep_unused=True)
    shard = NamedSharding(mesh, PartitionSpec("core"))
    zeros_fn = jax.jit(
        lambda: jnp.zeros((NCORES * NB, P, N), jnp.bfloat16),
        out_shardings=shard)
    # device-resident f16 zeros reused whenever ninf_mask is all-zero
    ninf_zero = jax.jit(lambda: jnp.zeros((NCORES * NB, P, N), jnp.float16),
                        out_shardings=shard)()
    jax.block_until_ready(ninf_zero)

    # weights travel as two small blobs to core 0 only (cores 1-7 get
    # device-made zeros); an on-device psum broadcasts, then local slices
    # rebuild each weight in the tiled P("core") layout the NEFF expects
    def _wunpack_body(b16, b32):
        b16 = jax.lax.psum(b16, "core")
        b32 = jax.lax.psum(b32, "core")
        outs = []
        for blob, spec in ((b16, WSPEC16), (b32, WSPEC32)):
            off = 0
            for _, shp in spec:
                n = int(np.prod(shp))
                outs.append(blob[:, off:off + n].reshape((1,) + shp))
                off += n
        return tuple(outs)

    nsp = len(WSPEC16) + len(WSPEC32)
    wunpack = jax.jit(shard_map(
        _wunpack_body, mesh=mesh,
        in_specs=(PartitionSpec("core"),) * 2,
        out_specs=(PartitionSpec("core"),) * nsp, check_rep=False))
    wz16 = jax.jit(lambda: jnp.zeros((NCORES, N16), jnp.float16),
                   out_shardings=shard)()
    wz32 = jax.jit(lambda: jnp.zeros((NCORES, N32), jnp.float32),
                   out_shardings=shard)()
    jax.block_until_ready([wz16, wz32])
    z16, z32 = [None] * NCORES, [None] * NCORES
    for s in wz16.addressable_shards:
        z16[devices.index(s.device)] = s.data
    for s in wz32.addressable_shards:
        z32[devices.index(s.device)] = s.data

    _STATE = {"nc": nc, "sharded": sharded, "in_names": in_names,
              "shard": shard, "zeros_fn": zeros_fn,
              "donate_bufs": [None] * CH, "ninf_zero": ninf_zero,
              "wunpack": wunpack, "z16": z16, "z32": z32,
              "devices": devices, "wnames": [n for n, _ in WSPEC16 + WSPEC32]}
    return _STATE


def _prep_weights(inp):
    w = {}
    lm_W1 = inp["lm_W1"]
    wl1t = np.zeros((4, 24, E), np.float32)
    for s in range(4):
        for a in range(2):
            for c in range(DYN):
                wl1t[s, 6 * s + 3 * a + c, 64 * a:64 * a + HID] = lm_W1[c]
    w["wl1t"] = wl1t
    w2d = np.zeros((E, E), np.float32)
    w2d[:HID, :HID] = inp["lm_W2"]
    w2d[HID:, HID:] = inp["lm_W2"]
    w["w2d"] = w2d
    w3d = np.zeros((E, 32), np.float32)
    w3d[:HID, 0] = inp["lm_W3"][:, 0]
    w3d[HID:, 1] = inp["lm_W3"][:, 0]
    w["w3d"] = w3d
    w["wq2"] = 2.0 * inp["Wq_last"][:E]
    w["wq2b"] = 2.0 * inp["Wq_last"][E:E + 1]
    w["fw1a"] = inp["film_W1"][:E, :E]
    w["fw1b"] = inp["film_W1"][E:E + 1, :E]
    w["fw1c"] = inp["film_W1"][:E, E:E + 1]
    w["fw1d"] = inp["film_W1"][E:E + 1, E:E + 1]
    w["fw2a"] = inp["film_W2"][:E]
    w["fw2b"] = inp["film_W2"][E:E + 1]
    w["fb1"] = inp["film_b1"][:E, None]
    w["fb1e"] = inp["film_b1"][E:E + 1, None]
    w["fb2"] = inp["film_b2"][:, None]
    w["lqw"] = inp["lazy_q_W"][:E]
    w["lqwb"] = inp["lazy_q_W"][E:E + 1]
    w["lqb"] = inp["lazy_q_b"][:, None]
    w["lmw1q"] = lm_W1[DYN:]
    w["lmb1"] = inp["lm_b1"][:, None]
    w["b2b2"] = np.concatenate([inp["lm_b2"], inp["lm_b2"]])[:, None]
    w["nb3"] = np.full((E, 1), -float(inp["lm_b3"][0]), np.float32)
    w["wk"] = inp["Wk"]
    w["wv"] = inp["Wv"]
    w["wcomb"] = inp["W_comb"]
    w["bcomb"] = inp["b_comb"][:, None]
    w["identm"] = np.eye(E, dtype=np.float32)
    # pack into one f16 blob + one f32 blob (per-core layout)
    b16 = np.empty((1, N16), np.float16)
    b32 = np.empty((1, N32), np.float32)
    for blob, spec in ((b16, WSPEC16), (b32, WSPEC32)):
        off = 0
        for k, shp in spec:
            n = int(np.prod(shp))
            blob[0, off:off + n] = np.asarray(w[k], dtype=blob.dtype).ravel()
            off += n
    return b16, b32


# lazy-MLP input layout: (P,N,3) -> 25 groups of 4 pomo rows, each row's
# 1000 nodes split in 2 halves of 500, features transposed to rows
def _xt_chunk(dyn, lo, hi):
    return np.ascontiguousarray(
        dyn[lo:hi].reshape(hi - lo, 25, 4, 2, 500, 3).transpose(0, 1, 2, 3, 5, 4)
    ).reshape(hi - lo, 25, 24, 500).astype(np.float16)


def kernel(**inputs):
    st = _get_state()
    inp = {k: np.asarray(v, dtype=np.float32) for k, v in inputs.items()}
    shard = st["shard"]
    devices = st["devices"]
    bpch = NCORES * NB              # batches per chunk

    # weights: small blobs to core 0, on-device broadcast + unpack
    b16, b32 = _prep_weights(inp)
    d16 = jax.device_put(b16, devices[0])
    d32 = jax.device_put(b32, devices[0])
    g16 = jax.make_array_from_single_device_arrays(
        (NCORES, N16), shard, [d16] + st["z16"][1:])
    g32 = jax.make_array_from_single_device_arrays(
        (NCORES, N32), shard, [d32] + st["z32"][1:])
    wdev = dict(zip(st["wnames"], st["wunpack"](g16, g32)))

    nodes_f16 = inp["encoded_nodes"].astype(np.float16)
    last_f16 = inp["encoded_last_node"].astype(np.float16)
    loadv = np.ascontiguousarray(inp["load"][:, None, :])
    ninf32 = inp["ninf_mask"]
    ninf_any = bool(ninf32.any())
    dyn = inp["dyn_features"]

    # chunk 0 is fully uploaded + dispatched before chunk 1's upload, so
    # chunk 0's exec + output fetch overlap chunk 1's transfers
    outs = []
    for ci in range(CH):
        lo, hi = ci * bpch, (ci + 1) * bpch
        g = {"nodes": jax.device_put(nodes_f16[lo:hi], shard),
             "last": jax.device_put(last_f16[lo:hi], shard),
             "loadv": jax.device_put(loadv[lo:hi], shard)}
        if ninf_any:
            g["ninf"] = jax.device_put(ninf32[lo:hi].astype(np.float16), shard)
        else:
            g["ninf"] = st["ninf_zero"]
        g["xt"] = jax.device_put(_xt_chunk(dyn, lo, hi), shard)
        g.update(wdev)
        args = [g[n] for n in st["in_names"]]
        donate_buf = st["donate_bufs"][ci]
        if donate_buf is None:
            donate_buf = st["zeros_fn"]()
        (o,) = st["sharded"](*args, donate_buf)
        st["donate_bufs"][ci] = o       # donated into the next call
        o.copy_to_host_async()
        outs.append(o)
    res = np.empty((B, P, N), np.float32)
    for ci, o in enumerate(outs):       # chunk 0 converts while chunk 1 runs
        res[ci * bpch:(ci + 1) * bpch] = np.asarray(o)
    return res
